# revision 12
# baseline (speedup 1.0000x reference)
"""Trainium2 Bass kernel for nn_BCE_Loss (retrieval_knn).

Distributed strategy (8 NeuronCores, SPMD):
  - Host: L2-normalize rows in f32, scale by 16 (keeps fp8 e4m3 in its normal
    range), cast to fp8_e4m3, pre-transpose to xT and lay out chunk-major
    [8 chunks, 128 D-partitions, 4 D-subtiles, 1024 rows]; each core's input
    is the chunk-rotated view so its own 1024 rows form chunk 0 (SPMD-uniform
    program, static diagonal position).
  - Device per core: 8 chunk DMAs (512KB each) land xT in SBUF; the
    [1024, 8192] similarity stripe is computed as 64 PSUM tiles [128, 1024]
    via fp8 DoubleRow matmuls (K=256 per instruction, 4 per tile; PSUM values
    are 256*cos). The self-match diagonal (static: local columns m*128..+127
    of column-block 0) is masked by subtracting 1000. ACT evacuates each PSUM
    tile to f16 SBUF; DVE takes top-8 per 1024-column block (max8) plus their
    indices (max_index). A final per-row-tile merge packs (round(v*8), col)
    into one f32 (q*8192 + col, exact in f32) and reduces the 64 candidates
    to sorted top-24 with 3 x (max8 + match_replace).
  - Host: decode packed top-24 -> (cos quantized to 2^-11, global column),
    gather labels, compute the BCE loss (tiny: 8192 x 20).

Approximations (validated in sim at rel err ~1.7e-4 vs the jax reference,
tolerance 2e-2): fp8 inputs perturb cos by ~2e-3 (rank swaps at the top-20
boundary are loss-neutral to ~1e-4); per-block top-8 union can miss a true
top-21 element only when >8 land in one block (~1e-3 of rows); f16 scan
values tie occasionally, duplicating a candidate index (loss-neutral since
tied values are equal to 1 f16 ulp).

Engine budget per core (warm 2.4GHz): PE 256 DoubleRow matmuls ~76us
(1.44x over bf16), ACT 64 evacuations ~55us, DVE 128 f16 scans + merges
~38us, DMA 4MB in ~11us. PE-bound.
"""

from contextlib import ExitStack

import numpy as np
import ml_dtypes

import concourse.bass as bass
import concourse.mybir as mybir
import concourse.tile as tile
from concourse.bass import ts
from concourse.bass_utils import run_bass_kernel_spmd
from concourse.vector_clock import ScopedClock, VectorClock

F32 = mybir.dt.float32
F16 = mybir.dt.float16
FP8 = mybir.dt.float8e4
U32 = mybir.dt.uint32
I32 = mybir.dt.int32
AF = mybir.ActivationFunctionType
ALU = mybir.AluOpType
DR = mybir.MatmulPerfMode.DoubleRow

B, D = 8192, 512
M = 8              # cores
BL = B // M        # 1024 rows per core
NRT = BL // 128    # 8 row tiles per core
NCB = 8            # 1024-col scan blocks
MAGIC = 12582912.0  # 1.5 * 2**23: adding+subtracting rounds to nearest int
BIGMAGIC = 103079215104.0  # 1.5 * 2**36: rounds v to the 2**13 grid
NEG = -3.0e38
EPS = 1e-12


# ---------------------------------------------------------------------------
# Environment workarounds: this container's walrus accepts at most ONE sem
# wait per instruction, and its runtime crashes on the explicit EventSemaphore
# butterfly barrier TileContext emits at its tail.
# ---------------------------------------------------------------------------

def _patched_drain_and_barrier(self, tick_clock, wait_clock):
    nc = self.nc
    vc = tick_clock.global_clock
    n = len(vc)
    for p in range(n):
        t = vc[p]
        if t > 0:
            pvc = VectorClock([0] * n)
            pvc.require_at_least(p, t)
            nop = nc.sync.nop()
            wait_clock.add_sem_waits(nop.ins, ScopedClock({None: pvc}))
    nc.sync.drain()
    nc._nrt_pseudo_barrier()
    assert self.sems is not None
    popped = nc._tile_sem_poison_stack.pop()
    assert popped is self._sem_poison
    nc.clear_and_free_semaphores(list(self.sems.allocated().values()))
    nc._nrt_pseudo_barrier()


tile.TileContext._drain_and_barrier = _patched_drain_and_barrier


def _split_multi_waits(nc):
    import bass_rust

    for f in nc.m.functions:
        for bb in f.blocks:
            out = []
            changed = False
            for ins in bb.instructions:
                si = ins.sync_info
                waits = list(si.on_wait) if si is not None else []
                if len(waits) > 1:
                    changed = True
                    for w in waits[:-1]:
                        nop = mybir.InstNoOp(
                            name=f"I-wsplit-{nc.next_id()}", ins=[], outs=[]
                        )
                        nop.engine = ins.engine
                        nop.sync_info = bass_rust.SyncInfo(on_wait=[w], on_update=[])
                        out.append(nop)
                    ins.sync_info = bass_rust.SyncInfo(
                        on_wait=[waits[-1]], on_update=list(si.on_update)
                    )
                out.append(ins)
            if changed:
                bb.instructions = out


# ---------------------------------------------------------------------------
# Kernel build
# ---------------------------------------------------------------------------

def build_nc(repeat=1):
    nc = bass.Bass(num_devices=M)
    xq = nc.declare_dram_parameter("xq", [8, 128, 4, BL], FP8, isOutput=False)
    out = nc.declare_dram_parameter("out", [BL, 24], F32, isOutput=True)
    for _rep in range(repeat):
        _build_body(nc, xq, out)
    _split_multi_waits(nc)
    return nc


def _build_body(nc, xq_dram, out):
    with tile.TileContext(nc) as tc, ExitStack() as octx:
        cpool = octx.enter_context(tc.tile_pool(name="const", bufs=1))
        # identity * 1000 for the diagonal (self-similarity) mask
        # (iota + compare, not affine_select: to_reg exhausts registers
        # when the body is chained many times for slope timing)
        it = cpool.tile([128, 128], I32)
        nc.gpsimd.iota(it[:], pattern=[[1, 128]], base=0,
                       channel_multiplier=-1)
        i1000 = cpool.tile([128, 128], F32)
        nc.vector.tensor_scalar(i1000[:], in0=it[:], scalar1=0,
                                scalar2=1000.0, op0=ALU.is_equal,
                                op1=ALU.mult)
        # off2[p, i] = 1024 * (i // 8) * 2^-25: scan-block base, pre-scaled
        off_i = cpool.tile([128, 64], I32)
        nc.gpsimd.iota(off_i[:], pattern=[[1024, 8], [0, 8]], base=0,
                       channel_multiplier=0)
        off2 = cpool.tile([128, 64], F32)
        nc.scalar.activation(off2[:], off_i[:], AF.Copy, scale=2.0 ** -25)
        # iotasc[p, j] = j * 2^-25 (local column, pre-scaled below the
        # 2^-12 psum grid so a plain f32 add packs value+column exactly)
        iota_i = cpool.tile([128, 1024], I32)
        nc.gpsimd.iota(iota_i[:], pattern=[[1, 1024]], base=0,
                       channel_multiplier=0)
        iotasc = cpool.tile([128, 1024], F32)
        nc.scalar.activation(iotasc[:], iota_i[:], AF.Copy, scale=2.0 ** -25)

        # xT chunks: [128 D-partitions, 4 D-subtiles, 1024 rows] fp8 each
        xt_pool = octx.enter_context(tc.tile_pool(name="xt", bufs=1))
        xt = [
            xt_pool.tile([128, 4, BL], FP8, tag=f"xt_{ch}", name=f"xt_{ch}")
            for ch in range(8)
        ]

        mm = octx.enter_context(tc.tile_pool(name="mm", bufs=4, space="PSUM"))
        sb = octx.enter_context(tc.tile_pool(name="sb", bufs=6))
        cand = octx.enter_context(tc.tile_pool(name="cand", bufs=1))
        fin = octx.enter_context(tc.tile_pool(name="fin", bufs=2))

        # split the 8 chunk loads across three DMA queues (qSP + qAct +
        # qPool SWDGE): one queue sustains only ~85 GB/s
        for ch in range(8):
            eng = (nc.sync, nc.scalar, nc.gpsimd)[ch % 3]
            eng.dma_start(xt[ch][:], xq_dram[ch])

        vals = [
            cand.tile([128, 64], F32, tag=f"VALS{m}", name=f"VALS{m}")
            for m in range(NRT)
        ]

        def do_block(pair, m):
            cbs = (2 * pair, 2 * pair + 1)
            pss = [
                mm.tile([128, 1024], F32, tag="ps", name=f"ps_{m}_{cb}")
                for cb in cbs
            ]
            # g-outer ordering: one weight load serves 4 matmuls
            for g in range(2):
                lhsT = xt[0][:, 2 * g:2 * g + 2, ts(m, 128)]
                for j, cb in enumerate(cbs):
                    for h in range(2):
                        nc.tensor.matmul(
                            pss[j][:, ts(h, 512)], lhsT,
                            xt[cb][:, 2 * g:2 * g + 2, ts(h, 512)],
                            start=(g == 0), stop=(g == 1),
                            perf_mode=DR,
                        )
            for j, cb in enumerate(cbs):
                # ACT evacuates PSUM (GPSIMD cannot access PSUM); values are
                # exact multiples of 2^-12 thanks to the integer-grid inputs
                sbt = sb.tile([128, 1024], F32, tag="sb")
                nc.scalar.copy(sbt[:], pss[j][:])
                # one plain add on the otherwise idle GPSIMD packs
                # value+column exactly (iota scaled below the psum grid)
                pkt = sb.tile([128, 1024], F32, tag="pk")
                nc.gpsimd.tensor_tensor(pkt[:], sbt[:], iotasc[:],
                                        op=ALU.add)
                if cb == 0:
                    # all diagonals live in local columns m*128..+127
                    o = m * 128
                    nc.gpsimd.tensor_tensor(
                        pkt[:, o:o + 128], pkt[:, o:o + 128],
                        i1000[:], op=ALU.subtract,
                    )
                # one f32 max8 pass gets value+index together
                nc.vector.max(vals[m][:, cb * 8:cb * 8 + 8], pkt[:])

        # add the scan-block base into the column field, then merge the 64
        # packed candidates to sorted top-24
        def do_merge(m):
            p0 = fin.tile([128, 64], F32, tag="p0")
            nc.gpsimd.tensor_tensor(p0[:], vals[m][:], off2[:], op=ALU.add)
            pv = fin.tile([128, 24], F32, tag="pv")
            p1 = fin.tile([128, 64], F32, tag="p1")
            p2 = fin.tile([128, 64], F32, tag="p2")
            nc.vector.max(pv[:, 0:8], p0[:])
            nc.vector.match_replace(p1[:], pv[:, 0:8], p0[:], NEG)
            nc.vector.max(pv[:, 8:16], p1[:])
            nc.vector.match_replace(p2[:], pv[:, 8:16], p1[:], NEG)
            nc.vector.max(pv[:, 16:24], p2[:])
            nc.sync.dma_start(out[ts(m, 128), :], pv[:])

        for pair in range(4):
            for m in range(NRT):
                do_block(pair, m)
        for m in range(NRT):
            do_merge(m)


_NC = None


def _get_nc():
    global _NC
    if _NC is None:
        _NC = build_nc()
    return _NC


def quantize(x32):
    """Normalize rows, quantize to the 1/64 integer grid (exact in e4m3)."""
    n = np.sqrt(np.einsum("ij,ij->i", x32, x32, dtype=np.float64))
    n = np.maximum(n, EPS).astype(np.float32)
    xh = x32 / n[:, None]
    a = np.clip(np.round(xh * 64.0), -16, 16).astype(np.float32)
    return a


def prep_inputs(x32, a=None):
    """Host prep: quantize, fp8 cast, transpose chunk-major, rotate."""
    if a is None:
        a = quantize(x32)
    x8 = (a / 64.0).astype(ml_dtypes.float8_e4m3)
    # C[ch, p, d4, t] = x8[ch*1024 + t, d4*128 + p]
    C = np.ascontiguousarray(x8.reshape(8, BL, 4, 128).transpose(0, 3, 2, 1))
    return [
        {"xq": np.ascontiguousarray(C[(np.arange(8) + c) % 8])}
        for c in range(M)
    ]


def run_device(x32, trace=False, **kwargs):
    """Run the SPMD kernel; returns (pv [B, 24] f32, BassKernelResults)."""
    nc = _get_nc()
    in_maps = prep_inputs(x32)
    res = run_bass_kernel_spmd(nc, in_maps, core_ids=list(range(M)),
                               trace=trace, **kwargs)
    pv = np.concatenate([res.results[c]["out"] for c in range(M)], axis=0)
    return pv, res


def decode_loss(pv, labels, k, a):
    """Decode packed top-24 -> (cos, global column ids) -> BCE loss.

    pv entries are S*2^-12 + col*2^-25 with S = <a_i, a_j> an integer.
    The device ranks by the raw quantized dot; the host divides by the true
    norms of the quantized vectors (known exactly) and re-sorts by cosine.
    """
    p64 = pv.astype(np.float64) * 2.0 ** 25
    S = np.floor(p64 / 8192.0)
    col = (p64 - S * 8192.0).astype(np.int64)        # local column in [0, 8192)
    core = np.arange(B) // BL                         # global row -> core
    gidx = (col + (core * BL)[:, None]) % B           # local -> global column
    nq = np.sqrt((a.astype(np.float64) ** 2).sum(1)) / 64.0
    vhat = (S / 4096.0) / (nq[:, None] * nq[gidx])    # corrected cosine
    o2 = np.argsort(-vhat, axis=1, kind="stable")[:, :k]
    vk = np.take_along_axis(vhat, o2, axis=1)
    ck = np.take_along_axis(gidx, o2, axis=1)
    preds = (vk + 1.0) * 0.5
    t = (labels[ck] == labels[:, None]).astype(np.float64)
    logp = np.maximum(np.log(preds), -100.0)
    log1mp = np.maximum(np.log1p(-preds), -100.0)
    loss = -(t * logp + (1.0 - t) * log1mp)
    return np.float32(loss.mean())


def kernel(batch, labels, k):
    k = int(k)
    assert 0 < k <= 24, f"kernel supports k <= 24, got {k}"
    x32 = np.asarray(batch, dtype=np.float32)
    assert x32.shape == (B, D)
    labels = np.asarray(labels)
    a = quantize(x32)
    nc = _get_nc()
    in_maps = prep_inputs(x32, a)
    res = run_bass_kernel_spmd(nc, in_maps, core_ids=list(range(M)))
    pv = np.concatenate([res.results[c]["out"] for c in range(M)], axis=0)
    return decode_loss(pv, labels, k, a)
